# revision 8
# baseline (speedup 1.0000x reference)
"""CrossAttention (linear-attention style) Trainium2 kernel.

Reference computation (per batch b, H=12 heads, d=64):
    kv  = seg @ W_kv                             # [N, 2C]
    ctx_h = softmax(scale * k_h^T v_h, axis=d)   # [d, d] per head
    o1_h = q1_h @ ctx_h ; o2_h = q2_h @ ctx_h    # [N, d]

Strategy: data-parallel over batch, one batch per NeuronCore (8 cores).
Per core, everything contracts over feature dims while activations are
n-major, so activations are transposed 128x128-tile-wise on the PE
(fp32 transpose-mode matmul vs identity); all real matmuls run as
float32r (full-rate fp32 mode) with fp32 PSUM accumulation.

ctx is computed transposed (ctxT[e,d] = sum_m v[m,e] k[m,d]) so the
softmax reduction lands on the free axis; packed per head-pair/quad to
keep the moving operand >=256 wide (full float32r rate). After the
softmax the 64x64 blocks are re-transposed into block-diagonal
[d-pair, e-quad] tiles consumed by the o-matmuls.

PSUM banks (8): 6 persistent pair-accumulators (tags ctx0..ctx5, one
accumulation group per bank) + 2 rotating transient banks (tags
trA/trB) shared by transpose groups and kv-projection chunks. In phase
2 the six ctx banks are recycled for the six o-quad groups per m-tile.
"""

import sys

for _p in ("/opt/trn_rl_repo",):
    if _p not in sys.path:
        sys.path.insert(0, _p)

import numpy as np

import concourse.bass as bass
import concourse.mybir as mybir
import concourse.tile as tile
from concourse import bacc
from concourse.masks import make_identity
from concourse.tile import add_dep_helper

B, N, C = 8, 4096, 768
H, D = 12, 64
SCALE = D ** -0.5
P = 128
MT = N // P          # 32 m-tiles
CS = C // P          # 6 column subtiles of 128
F32 = mybir.dt.float32
F32R = mybir.dt.float32r


def r(ap):
    """Reinterpret an fp32 AP as float32r for full-rate PE matmuls."""
    return ap.bitcast(F32R)


def _emit_transposes(nc, psum_pool, ident, src, dst):
    """PE-transpose src [128, 768] into dst [128, 6, 128] (c-major tiles).

    Batches the six 128x128 transposes into two PSUM banks (4 + 2) so
    the PSUM->SBUF eviction is two wide copies instead of six narrow
    ones.  Within a bank the transposes write disjoint quarters, so the
    scheduler sees no data deps between them; chain them explicitly so
    the start=True one really is first and stop=True last.
    """
    for tag, js in (("trA", range(0, 4)), ("trB", range(4, 6))):
        n = len(js)
        tp = psum_pool.tile([P, 512], F32, tag=tag, name=tag, bufs=1)
        prev = None
        for idx, j in enumerate(js):
            mm = nc.tensor.matmul(
                tp[:, idx * P:(idx + 1) * P],
                src[:, j * P:(j + 1) * P],
                ident,
                start=(idx == 0),
                stop=(idx == n - 1),
                is_transpose=True,
            )
            if prev is not None:
                add_dep_helper(mm.ins, prev.ins, sync=False,
                               reason="psum zero-region group order")
            prev = mm
        j0 = js[0]
        nc.any.tensor_copy(out=dst[:, j0:j0 + n, :], in_=tp[:, : n * P])


def build_kernel(nc, tc, seg, x1, x2, W, o1, o2):
    with (
        tc.tile_pool(name="persist", bufs=1) as persist,
        tc.tile_pool(name="work", bufs=1) as work,
        tc.tile_pool(name="psum", bufs=1, space="PSUM") as psum,
    ):
        # ---- constants / persistent state
        ident = persist.tile([P, P], F32, tag="ident")
        make_identity(nc, ident)
        W_s = persist.tile([P, CS, 2 * C], F32R, tag="Ws")
        for jc in range(6):
            w_stage = work.tile([P, CS, 256], F32, tag="wstage", bufs=2)
            nc.sync.dma_start(
                w_stage[:],
                W[:, jc * 256:(jc + 1) * 256].rearrange("(j p) f -> p j f", p=P),
            )
            nc.any.tensor_copy(
                out=W_s[:, :, jc * 256:(jc + 1) * 256], in_=w_stage[:]
            )
        ctxq = persist.tile([P, 6, 256], F32R, tag="ctxq")
        zsrc = work.tile([P, 6, 256], F32, tag="zsrc", bufs=1)
        nc.vector.memset(zsrc[:], 0.0)
        nc.vector.tensor_copy(out=ctxq[:], in_=zsrc[:])
        stg = []
        for p_ in range(6):
            t = persist.tile([P, P], F32, tag=f"stg{p_}", name=f"stg{p_}")
            nc.vector.memset(t[:], 0.0)
            stg.append(t)
        negmax = persist.tile([P, 6], F32, tag="negmax")
        sums = persist.tile([P, 6], F32, tag="sums")
        rec = persist.tile([P, 6], F32, tag="rec")

        # persistent ctxT accumulators: one PSUM bank per head-pair,
        # [e-pair(128) x d-quad(256)], accumulated over all 32 m-tiles
        ctx_ps = [
            psum.tile([P, 256], F32, tag=f"ctx{p_}", name=f"ctx{p_}", bufs=1)
            for p_ in range(6)
        ]

        # ---- phase 1: kv projection + ctxT accumulation over m
        for i in range(MT):
            seg_i = work.tile([P, C], F32, tag="seg", bufs=3)
            nc.sync.dma_start(seg_i[:], seg[i * P:(i + 1) * P, :])
            segT_i = work.tile([P, CS, P], F32R, tag="segT", bufs=2)
            _emit_transposes(nc, psum, ident, seg_i, segT_i)

            kv_i = work.tile([P, 2 * C], F32R, tag="kv", bufs=2)
            for jc in range(3):
                kv_ps = psum.tile([P, 512], F32, tag="trA", name="kvps", bufs=1)
                for j in range(CS):
                    nc.tensor.matmul(
                        kv_ps[:],
                        segT_i[:, j, :],
                        W_s[:, j, jc * 512:(jc + 1) * 512],
                        start=(j == 0),
                        stop=(j == CS - 1),
                    )
                nc.any.tensor_copy(
                    out=kv_i[:, jc * 512:(jc + 1) * 512], in_=kv_ps[:]
                )

            # ctxT accumulation: lhsT = v head-pair, rhs = k head-quad
            for p_ in range(6):
                q = p_ // 2
                nc.tensor.matmul(
                    ctx_ps[p_][:],
                    kv_i[:, C + P * p_: C + P * p_ + P],
                    kv_i[:, 256 * q: 256 * q + 256],
                    start=(i == 0),
                    stop=(i == MT - 1),
                )

        # ---- softmax over d (free axis of ctxT blocks), then transpose back
        def blk(h):
            p_ = h // 2
            rows = slice((h % 2) * 64, (h % 2) * 64 + 64)
            col0 = (h - 4 * (p_ // 2)) * 64
            return ctx_ps[p_][rows, col0:col0 + 64], p_, rows

        for h in range(H):
            block, p_, rows = blk(h)
            nc.vector.reduce_max(
                negmax[rows, p_:p_ + 1], block, axis=mybir.AxisListType.X
            )
        nc.vector.tensor_scalar_mul(negmax[:], negmax[:], -SCALE)
        for h in range(H):
            block, p_, rows = blk(h)
            r0 = (h % 2) * 64
            nc.scalar.activation(
                stg[p_][rows, r0:r0 + 64],
                block,
                mybir.ActivationFunctionType.Exp,
                bias=negmax[rows, p_:p_ + 1],
                scale=SCALE,
                accum_out=sums[rows, p_:p_ + 1],
            )
        nc.vector.reciprocal(rec[:], sums[:])
        for h in range(H):
            _, p_, rows = blk(h)
            r0 = (h % 2) * 64
            nc.scalar.activation(
                stg[p_][rows, r0:r0 + 64],
                stg[p_][rows, r0:r0 + 64],
                mybir.ActivationFunctionType.Copy,
                scale=rec[rows, p_:p_ + 1],
            )
        for p_ in range(6):
            tp = psum.tile([P, 512], F32, tag="trB", name="ctxT", bufs=1)
            nc.tensor.matmul(tp[:, :P], stg[p_], ident, start=True, stop=True,
                             is_transpose=True)
            nc.any.tensor_copy(
                out=ctxq[:, p_, (p_ % 2) * P:(p_ % 2) * P + P], in_=tp[:, :P]
            )

        # ---- phase 2: o1/o2 = q @ ctx (block-diagonal), per m-tile
        for i in range(MT):
            for t_, (name, x, o) in enumerate((("x1", x1, o1), ("x2", x2, o2))):
                x_i = work.tile([P, C], F32, tag=f"{name}", bufs=6)
                nc.sync.dma_start(x_i[:], x[i * P:(i + 1) * P, :])
                xT_i = work.tile([P, CS, P], F32R, tag=f"{name}T", bufs=2)
                _emit_transposes(nc, psum, ident, x_i, xT_i)

                o_i = work.tile([P, C], F32, tag=f"o_{name}", bufs=3)
                for q in range(3):
                    tag = f"ctx{3 * t_ + q}"
                    o_ps = psum.tile([P, 256], F32, tag=tag, name=f"o{tag}",
                                     bufs=1)
                    for k, p_ in enumerate((2 * q, 2 * q + 1)):
                        nc.tensor.matmul(
                            o_ps[:],
                            xT_i[:, p_, :],
                            ctxq[:, p_, :],
                            start=(k == 0),
                            stop=(k == 1),
                        )
                    nc.any.tensor_copy(
                        out=o_i[:, q * 256:(q + 1) * 256], in_=o_ps[:]
                    )
                nc.sync.dma_start(o[i * P:(i + 1) * P, :], o_i[:])


def build_nc():
    nc = bacc.Bacc(None, target_bir_lowering=False)
    seg = nc.declare_dram_parameter("seg", [N, C], F32, isOutput=False)
    x1 = nc.declare_dram_parameter("x1", [N, C], F32, isOutput=False)
    x2 = nc.declare_dram_parameter("x2", [N, C], F32, isOutput=False)
    W = nc.declare_dram_parameter("W", [C, 2 * C], F32, isOutput=False)
    o1 = nc.declare_dram_parameter("o1", [N, C], F32, isOutput=True)
    o2 = nc.declare_dram_parameter("o2", [N, C], F32, isOutput=True)
    with tile.TileContext(nc) as tc:
        build_kernel(nc, tc, seg, x1, x2, W, o1, o2)
    nc.finalize()
    return nc


_NC_CACHE = None


def kernel(x1, x2, segfeature, W_kv, **_ignored):
    global _NC_CACHE
    from concourse.bass_utils import run_bass_kernel_spmd

    if _NC_CACHE is None:
        _NC_CACHE = build_nc()
    nc = _NC_CACHE

    x1 = np.ascontiguousarray(np.asarray(x1, dtype=np.float32))
    x2 = np.ascontiguousarray(np.asarray(x2, dtype=np.float32))
    seg = np.ascontiguousarray(np.asarray(segfeature, dtype=np.float32))
    W = np.ascontiguousarray(np.asarray(W_kv, dtype=np.float32))

    in_maps = [
        {"seg": seg[b], "x1": x1[b], "x2": x2[b], "W": W} for b in range(B)
    ]
    res = run_bass_kernel_spmd(nc, in_maps, core_ids=list(range(B)))
    o1 = np.stack([res.results[b]["o1"] for b in range(B)])
    o2 = np.stack([res.results[b]["o2"] for b in range(B)])
    return o1, o2
